# revision 1
# baseline (speedup 1.0000x reference)
"""Trainium2 Bass kernel for nn_Conv2d_68298569941797.

Conv2d: data [32,1,224,224] f32 (x) weight [64,1,3,3] f32 -> out [32,64,222,222] f32
(valid padding, stride 1, cross-correlation).

Strategy (data-parallel over batch, 4 images per NeuronCore x 8 cores):
  The conv is lowered to a single stationary matmul per output chunk.
  Output rows are split into two halves (0..110 / 111..221). The stationary
  operand lhsT is [K=18, M=128]: K = (half, ky, kx), M = (half, out_channel),
  with zeros in the cross-half blocks. The moving operand rhs [18, N] is read
  from 18 shifted copies of the image resident in SBUF: partition
  k = (h, ky, kx) holds the image shifted by (111*h + ky) rows and kx cols.
  One matmul column computes all 128 = 2x64 outputs for one output pixel pair
  ((y, x) for half 0 and (y+111, x) for half 1).

  Chunks: 2 output rows x 222 cols = 444 columns per matmul (fits one PSUM
  bank, and N>=256 keeps float32r matmul at 1 cycle/row). 4 chunks stage into
  one SBUF tile [128, 1776] whose free dim maps to 8 contiguous output rows,
  so the output DMA writes contiguous 7104B runs per (half, channel).

This file is self-contained: shapes/sharding are hardcoded; it only imports
installed packages (numpy, concourse).
"""

import numpy as np

import concourse.bass as bass
import concourse.mybir as mybir
import concourse.tile as tile
from concourse import bacc
from concourse.bass_utils import run_bass_kernel_spmd

N_CORES = 8
B, H, W = 32, 224, 224
O, KH, KW = 64, 3, 3
OH, OW = 222, 222
BPC = B // N_CORES          # images per core
HALF = OH // 2              # 111 output rows per half
KP = 18                     # contraction: (half, ky, kx)
M = 128                     # outputs per column: (half, out_channel)
SEG = 111 * W               # 24864: contiguous elems loaded per partition
IMG = H * W                 # 50176
DATA_LEN = BPC * IMG + 2    # flat padded per-core input (+2: shift-window tail)
OIMG = O * OH * OW          # per-image output elems
CHUNK_ROWS = 2              # output rows per matmul chunk
CHUNK_N = CHUNK_ROWS * OW   # 444 matmul columns
BLK_CHUNKS = 8              # chunks per staged output DMA
BLK_N = BLK_CHUNKS * CHUNK_N
# block base rows: 6 blocks of 16 rows + one final overlapping block
BLOCK_YS = [16 * j for j in range(6)] + [95]

MM_DT = mybir.dt.float32r


def _build_body(tc, data_ap, weight_ap, out_ap, reps=1, variant="full"):
    nc = tc.nc
    data_t = data_ap.tensor
    weight_t = weight_ap.tensor
    out_t = out_ap.tensor
    do_in = variant not in ("noin",)
    do_mm = variant not in ("nocompute", "dmaonly")
    do_out = variant not in ("noout",)

    with (
        tc.tile_pool(name="const", bufs=1) as const_pool,
        tc.tile_pool(name="imgp", bufs=1) as img_pool,
        tc.tile_pool(name="psp", bufs=8, space="PSUM") as psum_pool,
        tc.tile_pool(name="stp", bufs=3) as stage_pool,
    ):
        # lhsT [18, 128]: host-prescattered (see make_in_maps), loaded with a
        # single SWDGE DMA that casts f32 -> f32r (the fast fp32 matmul
        # format; producers of f32r-consumed data must write f32r).
        lhsT = const_pool.tile([KP, M], MM_DT)
        nc.sync.dma_start(lhsT[:], bass.AP(weight_t, 0, [[M, KP], [1, M]]))

        for b in [b for _ in range(reps) for b in range(BPC)]:
            # 18 shifted image copies; partition k=(h,ky,kx) holds the
            # contiguous window data[b].flat[(111h+ky)*224+kx :][:SEG]
            img3 = img_pool.tile([KP, 111, W], MM_DT)
            if do_in:
                # 4 HWDGE loads (ACT ring): [9 partitions, band] each; the
                # 9 shifted copies come from (ky, kx) source dims.
                for h in range(2):
                    for r0, R in ((0, 56), (56, 55)):
                        src = bass.AP(
                            data_t, b * IMG + (HALF * h + r0) * W,
                            [[W, 3], [1, 3], [1, R * W]],
                        )
                        nc.scalar.dma_start(
                            img3[h * 9:(h + 1) * 9, r0:r0 + R, :], src)

            for Y in BLOCK_YS:
                stage = stage_pool.tile([M, BLK_N], mybir.dt.float32)
                if variant == "dmaonly":
                    nc.gpsimd.memset(stage[:], 0)
                if do_mm:
                    for i in range(BLK_CHUNKS):
                        y0 = Y + CHUNK_ROWS * i
                        ps = psum_pool.tile([M, CHUNK_N], mybir.dt.float32)
                        rhs = img3[:, y0:y0 + CHUNK_ROWS, 0:OW]
                        nc.tensor.matmul(
                            ps[:], lhsT[:], rhs,
                            start=True, stop=True,
                        )
                        nc.vector.tensor_copy(
                            stage[:, i * CHUNK_N:(i + 1) * CHUNK_N], ps[:])
                if do_out:
                    # stage free dim = 16 contiguous output rows starting at Y
                    dest = bass.AP(
                        out_t, b * OIMG + Y * OW,
                        [[HALF * OW, 2], [OH * OW, 64], [1, BLK_N]],
                    )
                    nc.sync.dma_start(dest, stage[:])


_NC_CACHE = {}


def _get_nc(reps=1, variant="full"):
    key = (reps, variant)
    if key not in _NC_CACHE:
        nc = bacc.Bacc(
            "TRN2",
            target_bir_lowering=False,
            debug=False,
            num_devices=N_CORES,
        )
        data = nc.dram_tensor(
            "data", [DATA_LEN], MM_DT, kind="ExternalInput").ap()
        weight = nc.dram_tensor(
            "weight", [KP, M], MM_DT,
            kind="ExternalInput").ap()
        out = nc.dram_tensor(
            "out", [BPC, O, OH, OW], mybir.dt.float32,
            kind="ExternalOutput").ap()
        with tile.TileContext(nc) as tc:
            _build_body(tc, data, weight, out, reps=reps, variant=variant)
        nc.compile()
        _NC_CACHE[key] = nc
    return _NC_CACHE[key]


def make_in_maps(data, weight):
    data = np.ascontiguousarray(np.asarray(data), dtype=np.float32)
    weight = np.ascontiguousarray(np.asarray(weight), dtype=np.float32)
    # host-side scatter of w[o,0,ky,kx] into lhsT [K=(h,ky,kx), M=(h,o)]
    lhsT = np.zeros((KP, M), np.float32)
    blk = weight[:, 0].transpose(1, 2, 0).reshape(9, O)  # [(ky,kx), o]
    for h in range(2):
        lhsT[h * 9:(h + 1) * 9, h * O:(h + 1) * O] = blk
    in_maps = []
    for c in range(N_CORES):
        flat = data[c * BPC:(c + 1) * BPC].reshape(-1)
        flat = np.concatenate([flat, np.zeros(2, np.float32)])
        in_maps.append({"data": flat, "weight": lhsT})
    return in_maps


def kernel(data, weight):
    nc = _get_nc()
    res = run_bass_kernel_spmd(
        nc, make_in_maps(data, weight), core_ids=list(range(N_CORES)))
    return np.concatenate([r["out"] for r in res.results], axis=0)



# revision 18
# speedup vs baseline: 184.9833x; 184.9833x over previous
"""Trainium2 Bass kernel for nn_Conv2d_68298569941797.

Conv2d: data [32,1,224,224] f32 (x) weight [64,1,3,3] f32 -> out [32,64,222,222] f32
(valid padding, stride 1, cross-correlation).

Data-parallel over batch: 4 images per NeuronCore x 8 cores.

Per-core formulation (stationary matmul, all 128 PE output columns used):
  lhsT [K=18, M=128]: K = (half, ky, kx), M = (half, out_channel), zeros in
  the cross-half blocks (host-prescattered). The moving operand rhs [18, 448]
  streams from an im2col buffer of 18 shifted image copies.

  Output is computed over a 112x224 per-(half,channel) grid (224 cols incl. 2
  garbage cols per row, 112 rows incl. 1 garbage row) so that every chunk is a
  uniform [18, 448] contiguous slice and each partition's per-image output is
  one contiguous 25088-element run in DRAM. The host strips the garbage when
  unsharding. 56 chunks of 2 rows cover an image.

  im2col: output rows are split into 7 groups of 16; group g needs image rows
  16g+ky.. (+16) per (half, ky, kx). Groups are packed 3-per-tile at partition
  bases {0, 32, 64} (matmul operands must start at partition 0/32/64), with 3
  copies of lhsT at the same bases. Each group's partition p = base + h*9 +
  ky*3 + kx holds flat image elems [(111h + 16g + ky)*224 + kx : +3584].
  Only ~43KB/partition per image -> double-buffered for cross-image prefetch.

  Chunks drain PSUM -> SBUF stage alternating between DVE and ACT engines.
  Stage [128, 6272] covers 28 output rows; one SWDGE (gpsimd) dma_start per
  stage writes 3.2MB with 128 x 25KB descriptors, spreading across all 16
  SDMA engines (the HWDGE dynamic rings only engage 2-3 engines, ~27GB/s
  each, which was the previous bottleneck).

Self-contained: shapes/sharding hardcoded; imports only installed packages.
"""

import numpy as np

import concourse.bass as bass
import concourse.mybir as mybir
import concourse.tile as tile
from concourse import bacc
from concourse.bass_utils import run_bass_kernel_spmd

N_CORES = 8
B, H, W = 32, 224, 224
O, KH, KW = 64, 3, 3
OH, OW = 222, 222
BPC = B // N_CORES          # images per core
HALF = 111                  # output rows per half
KP = 18                     # contraction: (half, ky, kx)
M = 128                     # outputs per column: (half, out_channel)
IMG = H * W                 # 50176
DATA_LEN = BPC * IMG + 228  # flat padded per-core input

NGRP = 4                    # row-groups of 28 output rows (4*28=112 incl garbage)
GROWS = 28
GSEG = GROWS * W            # 3584 elems per im2col partition
GPT = 3                     # groups per im2col tile (partition bases 0/32/64)
PROW = 112                  # padded out rows computed per (half,channel)
PCOL = 224                  # padded out cols (222 valid + 2 garbage)
VROW = 111                  # valid out rows actually written to DRAM
PIMG = VROW * PCOL          # 24864 contiguous elems per (b, partition)

CHUNK_ROWS = 2
CHUNK_N = CHUNK_ROWS * PCOL  # 448 matmul columns (<=512 f32 PSUM bank)
STAGE_ROWS = 112             # whole image staged per DMA (max descriptor size)
STAGE_N = STAGE_ROWS * PCOL  # 25088
STAGE_CHUNKS = STAGE_ROWS // CHUNK_ROWS  # 56
NSTAGE = PROW // STAGE_ROWS  # 1 output DMA per image

MM_DT = mybir.dt.float16
OUT_DT = mybir.dt.float16


def _build_body(tc, data_ap, weight_ap, out_ap, reps=1, variant="full"):
    nc = tc.nc
    data_t = data_ap.tensor
    weight_t = weight_ap.tensor
    out_t = out_ap.tensor

    seq = [b for _ in range(reps) for b in range(BPC)]
    # partition sizes of the 2 im2col tiles (groups 0-2, 3)
    tile_parts = [82, 18]

    with (
        tc.tile_pool(name="const", bufs=1) as const_pool,
        tc.tile_pool(name="imgp", bufs=2) as img_pool,
        tc.tile_pool(name="psp", bufs=8, space="PSUM") as psum_pool,
        tc.tile_pool(name="stp", bufs=3) as stage_pool,
    ):
        # lhsT [18, 128] host-prescattered (f32 bits viewed as f32r), copied
        # to partition bases 0/32/64 to pair with any im2col group base.
        lhsT3 = const_pool.tile([82, M], MM_DT)
        for base in (0, 32, 64):
            nc.sync.dma_start(
                lhsT3[base:base + KP, :], bass.AP(weight_t, 0, [[M, KP], [1, M]]))

        img_tiles = {}

        def load_img(i):
            b = seq[i]
            tiles = []
            for t_i, parts in enumerate(tile_parts):
                t = img_pool.tile([parts, GSEG], MM_DT,
                                  name=f"img_{i}_{t_i}", uniquify=False,
                                  tag=f"imgt{t_i}")
                tiles.append(t)
                for gi in range(parts // 32 + 1):
                    g = t_i * GPT + gi
                    for h in range(2):
                        src = bass.AP(
                            data_t, b * IMG + (HALF * h + GROWS * g) * W,
                            [[W, 3], [1, 3], [1, GSEG]],
                        )
                        p0 = gi * 32 + h * 9
                        nc.gpsimd.dma_start(t[p0:p0 + 9, :], src)
            img_tiles[i] = tiles

        load_img(0)
        for i, b in enumerate(seq):
            tiles = img_tiles.pop(i)
            if i + 1 < len(seq):
                # prefetch next image: enqueued on the Pool queue BEFORE this
                # image's big output DMA so its descriptors drain ahead of it;
                # the imgp WAR dep (image i-1's matmuls) is already satisfied
                # at this point in the queue, so desc-gen never blocks.
                load_img(i + 1)
            for s in range(NSTAGE):
                stage = stage_pool.tile([M, STAGE_N], OUT_DT)
                for j in range(STAGE_CHUNKS):
                    y0 = s * STAGE_ROWS + j * CHUNK_ROWS
                    g = y0 // GROWS
                    base = (g % GPT) * 32
                    off = (y0 - g * GROWS) * W
                    ps = psum_pool.tile([M, CHUNK_N], mybir.dt.float32)
                    rhs = tiles[g // GPT][base:base + KP, off:off + CHUNK_N]
                    nc.tensor.matmul(ps[:], lhsT3[base:base + KP, :], rhs,
                                     start=True, stop=True)
                    dst = stage[:, j * CHUNK_N:(j + 1) * CHUNK_N]
                    if j % 2 == 0:
                        nc.vector.tensor_copy(dst, ps[:])
                    else:
                        nc.scalar.copy(dst, ps[:])
                # one SWDGE DMA: 128 descriptors x ~50KB, all 16 SDMA engines.
                # The final stage is truncated to skip the garbage row 111.
                ssz = min(STAGE_N, PIMG - s * STAGE_N)
                dest = bass.AP(
                    out_t, b * M * PIMG + s * STAGE_N,
                    [[PIMG, M], [1, ssz]],
                )
                nc.gpsimd.dma_start(dest, stage[:, :ssz])


_NC_CACHE = {}


def _get_nc(reps=1, variant="full"):
    key = (reps, variant)
    if key not in _NC_CACHE:
        nc = bacc.Bacc(
            "TRN2",
            target_bir_lowering=False,
            debug=False,
            num_devices=N_CORES,
        )
        data = nc.dram_tensor(
            "data", [DATA_LEN], MM_DT, kind="ExternalInput").ap()
        weight = nc.dram_tensor(
            "weight", [KP, M], MM_DT,
            kind="ExternalInput").ap()
        out = nc.dram_tensor(
            "out", [BPC, M, VROW, PCOL], OUT_DT,
            kind="ExternalOutput").ap()
        with tile.TileContext(nc) as tc:
            _build_body(tc, data, weight, out, reps=reps, variant=variant)
        nc.compile()
        _NC_CACHE[key] = nc
    return _NC_CACHE[key]


def make_in_maps(data, weight):
    data = np.ascontiguousarray(np.asarray(data), dtype=np.float32)
    weight = np.ascontiguousarray(np.asarray(weight), dtype=np.float32)
    # host-side scatter of w[o,0,ky,kx] into lhsT [K=(h,ky,kx), M=(h,o)]
    lhsT = np.zeros((KP, M), np.float32)
    blk = weight[:, 0].transpose(1, 2, 0).reshape(9, O)  # [(ky,kx), o]
    for h in range(2):
        lhsT[h * 9:(h + 1) * 9, h * O:(h + 1) * O] = blk
    in_maps = []
    for c in range(N_CORES):
        flat = data[c * BPC:(c + 1) * BPC].reshape(-1)
        flat = np.concatenate([flat, np.zeros(DATA_LEN - flat.size, np.float32)])
        in_maps.append({"data": flat.astype(np.float16),
                        "weight": lhsT.astype(np.float16)})
    return in_maps


def kernel(data, weight):
    nc = _get_nc()
    res = run_bass_kernel_spmd(
        nc, make_in_maps(data, weight), core_ids=list(range(N_CORES)))
    full = np.empty((B, O, OH, OW), np.float32)
    for c, r in enumerate(res.results):
        o = r["out"].astype(np.float32)  # [BPC, 128, 111, 224] fp16
        full[c * BPC:(c + 1) * BPC, :, :HALF, :] = o[:, :O, :, :OW]
        full[c * BPC:(c + 1) * BPC, :, HALF:, :] = o[:, O:, :, :OW]
    return full
